# revision 6
# baseline (speedup 1.0000x reference)
"""Distributed causal self-attention for 8 Trainium2 NeuronCores.

Problem: x[2,2048,1024] @ w_qkv[1024,3072] -> causal MHA (16 heads, d=64)
         -> @ w_out[1024,1024]. All fp32 in/out.

Sharding: core c (0..7) handles batch b=c//4 and head group g=c%4 (4 heads).
Each core projects qkv for its heads (inputs host-cast to bf16), runs flash
attention with transposed-score layout (keys on partitions), then a per-
query-chunk AllToAll over all 8 cores redistributes attention outputs from
head-parallel to token-parallel; each core then applies the full out
projection for its 128-token slice of the chunk.

All matmuls bf16 (fp32 PSUM accumulation); scores use zero-padded
128-contraction (64-contraction matmuls are ~1.8x slower per column on
TRN2). Softmax exp on the ACT engine in fp32->bf16.
"""

import sys

for _p in ("/opt/trn_rl_repo", "/root/.axon_site/_ro/trn_rl_repo"):
    if _p not in sys.path:
        sys.path.insert(0, _p)

import ml_dtypes
import numpy as np

import concourse.bass as bass  # noqa: F401
import concourse.mybir as mybir
import concourse.tile as tile
from concourse import bacc
from concourse.bass_utils import run_bass_kernel_spmd

P = 128
B, T, C = 2, 2048, 1024
H, D = 16, 64
HL = 4               # heads per core
DL = HL * D          # 256 local head dims
KC = C // P          # 8 contraction tiles over C
QB = 512             # query chunk
NQ = T // QB         # 4 query chunks
NT = T // P          # 16 token tiles
G = 4                # cores per batch group
SCALE = 1.0 / 8.0    # 1/sqrt(64)
NEG = -1.0e30

F32 = mybir.dt.float32
BF16 = mybir.dt.bfloat16
BF = ml_dtypes.bfloat16

_CACHED = {}


def _mask_data():
    # tril mask: 0 where key j <= query i, NEG above the diagonal
    j = np.arange(P)[:, None]
    i = np.arange(P)[None, :]
    return np.where(j <= i, 0.0, NEG).astype(np.float32)


def _build(dbg=False):
    nc = bacc.Bacc("TRN2", target_bir_lowering=False, debug=False,
                   num_devices=8)

    # all weights/activations host-cast to bf16
    xT = nc.dram_tensor("xT", [C, T], BF16, kind="ExternalInput")
    wq = nc.dram_tensor("wq", [C, DL], BF16, kind="ExternalInput")
    wk = nc.dram_tensor("wk", [C, DL], BF16, kind="ExternalInput")
    wv = nc.dram_tensor("wv", [C, DL], BF16, kind="ExternalInput")
    bq = nc.dram_tensor("bq", [1, DL], F32, kind="ExternalInput")
    bk = nc.dram_tensor("bk", [1, DL], F32, kind="ExternalInput")
    bv = nc.dram_tensor("bv", [1, DL], BF16, kind="ExternalInput")
    wo = nc.dram_tensor("wo", [C, C], BF16, kind="ExternalInput")  # permuted
    bo = nc.dram_tensor("bo", [1, C], F32, kind="ExternalInput")
    sel = nc.dram_tensor("sel", [1, 2], F32, kind="ExternalInput")
    out = nc.dram_tensor("out", [NQ, P, C], F32, kind="ExternalOutput")
    if dbg:
        dbg_q = nc.dram_tensor("dbg_q", [P, 2, T], BF16, kind="ExternalOutput")
        dbg_k = nc.dram_tensor("dbg_k", [P, 2, 2, T], BF16,
                               kind="ExternalOutput")
        dbg_v = nc.dram_tensor("dbg_v", [P, NT, HL * (D + 1)], BF16,
                               kind="ExternalOutput")
        dbg_ai = nc.dram_tensor("dbg_ai", [NQ, 2 * G * 2 * P, P], BF16,
                                kind="ExternalOutput")
        dbg_ao = nc.dram_tensor("dbg_ao", [NQ, 2 * G * 2 * P, P], BF16,
                                kind="ExternalOutput")

    masks_dram = nc.inline_tensor(_mask_data(), name="cmasks")

    with tile.TileContext(nc) as tc:
        with (
            tc.tile_pool(name="const", bufs=1) as cp,
            tc.tile_pool(name="persist", bufs=1) as pp,
            tc.tile_pool(name="work", bufs=3) as wk_p,
            tc.tile_pool(name="dram", bufs=1, space="DRAM") as dp,
        ):
            # ---- small constants (issued first so weights DMA next) ----
            bq_col = cp.tile([P, 2], F32)
            bk_col = cp.tile([P, 2], F32)
            nc.sync.dma_start(
                bq_col[:], bq[0, :].rearrange("(m p) -> p m", p=P))
            nc.sync.dma_start(
                bk_col[:], bk[0, :].rearrange("(m p) -> p m", p=P))
            bv_sb = cp.tile([1, DL], BF16)
            nc.sync.dma_start(bv_sb[:], bv[:])
            ones_bf = cp.tile([1, P], BF16)
            nc.vector.memset(ones_bf[:], 1.0)

            # ---- weights then x (phase A starts as soon as tile 0 lands) --
            wq_sb = pp.tile([P, KC, DL], BF16)
            wk_sb = pp.tile([P, KC, DL], BF16)
            wv_sb = pp.tile([P, KC, DL], BF16)
            nc.sync.dma_start(wq_sb[:], wq.rearrange("(k p) m -> p k m", p=P))
            nc.sync.dma_start(wk_sb[:], wk.rearrange("(k p) m -> p k m", p=P))
            nc.sync.dma_start(wv_sb[:], wv.rearrange("(k p) m -> p k m", p=P))

            masks = cp.tile([P, P], F32)
            nc.sync.dma_start(masks[:], masks_dram[:])
            sel_sb = cp.tile([1, 2], F32)
            nc.sync.dma_start(sel_sb[:], sel[:])
            selA = cp.tile([P, 1], F32)
            selB = cp.tile([P, 1], F32)
            nc.gpsimd.partition_broadcast(selA[:], sel_sb[:, 0:1])
            nc.gpsimd.partition_broadcast(selB[:], sel_sb[:, 1:2])
            bo_sb = cp.tile([1, C], F32)
            nc.sync.dma_start(bo_sb[:], bo[:])
            bo_bc = cp.tile([P, C], F32)
            nc.gpsimd.partition_broadcast(bo_bc[:], bo_sb[:])

            # ---- persistent activations (bf16) ----
            # qT_pair[mi]: [128 = even-head d | odd-head d, tokens]
            qT_sb = pp.tile([P, 2, T], BF16)
            # kT_pad[parity]: zero-padded to 128 contraction rows
            kT_sb = pp.tile([P, 2, 2, T], BF16)   # [p, parity, mi, t]
            v_sb = pp.tile([P, NT, HL * (D + 1)], BF16)  # per head: 64 v + 1

            nc.vector.memset(kT_sb[64:P, 0], 0.0)
            nc.vector.memset(kT_sb[0:64, 1], 0.0)
            vones = v_sb.rearrange("p n (h e) -> p n h e", h=HL)[:, :, :,
                                                               D:D + 1]
            nc.vector.memset(vones, 1.0)

            with tc.tile_pool(name="xw", bufs=1) as xw:
                xb = xw.tile([P, KC, T], BF16)
                for kk in range(KC):
                    nc.sync.dma_start(
                        xb[:, kk, :],
                        xT.rearrange("(k p) t -> k p t", p=P)[kk])

                # ---- phase A: qkv projection, kk-major over 8 psum banks --
                with tc.tile_pool(name="ps_a", bufs=8, space="PSUM") as psa:
                    # stage 1+2: q/k for ni pairs, kk-major streaming vs DMA
                    for nh in range(2):          # ni half: (0,1) then (2,3)
                        groups = []
                        for (w_sb, b_col, dst) in ((wq_sb, bq_col, 0),
                                                   (wk_sb, bk_col, 1)):
                            for mi in range(2):
                                for ni in (2 * nh, 2 * nh + 1):
                                    groups.append((w_sb, b_col, dst, mi, ni))
                        tiles = {}
                        for gi, (w_sb, b_col, dst, mi, ni) in enumerate(groups):
                            tiles[gi] = psa.tile([P, QB], F32, name="ps_qk",
                                                 tag="ps_a")
                        for kk in range(KC):
                            for gi, (w_sb, b_col, dst, mi, ni) in enumerate(
                                    groups):
                                nc.tensor.matmul(
                                    tiles[gi][:],
                                    w_sb[:, kk, mi * P:(mi + 1) * P],
                                    xb[:, kk, ni * QB:(ni + 1) * QB],
                                    start=(kk == 0), stop=(kk == KC - 1))
                        for gi, (w_sb, b_col, dst, mi, ni) in enumerate(
                                groups):
                            ps = tiles[gi]
                            tsl = slice(ni * QB, (ni + 1) * QB)
                            if dst == 0:     # q: straight copy, pair-stacked
                                nc.vector.tensor_scalar_add(
                                    qT_sb[:, mi, tsl], ps[:],
                                    b_col[:, mi:mi + 1])
                            else:            # k: split halves into padded kT
                                nc.vector.tensor_scalar_add(
                                    kT_sb[0:64, 0, mi, tsl], ps[0:64, :],
                                    b_col[0:64, mi:mi + 1])
                                nc.vector.tensor_scalar_add(
                                    kT_sb[64:P, 1, mi, tsl], ps[64:P, :],
                                    b_col[64:P, mi:mi + 1])
                    # stage 3: v, [tok, dims] layout; one group per psum bank
                    # (start=True zeroes the whole bank, so never co-locate
                    # two accumulation groups in one bank)
                    for vp in range(2):
                        vt = {}
                        for sub in range(NT // 2):
                            vt[sub] = psa.tile([P, QB], F32, name="ps_v",
                                               tag="ps_a")
                        for kk in range(KC):
                            for sub in range(NT // 2):
                                ti = vp * (NT // 2) + sub
                                nc.tensor.matmul(
                                    vt[sub][:, 0:DL],
                                    xb[:, kk, ti * P:(ti + 1) * P],
                                    wv_sb[:, kk, :],
                                    start=(kk == 0), stop=False)
                        for sub in range(NT // 2):
                            nc.tensor.matmul(
                                vt[sub][:, 0:DL],
                                ones_bf[:, 0:P], bv_sb[:],
                                start=False, stop=True)
                        for sub in range(NT // 2):
                            ti = vp * (NT // 2) + sub
                            nc.vector.tensor_copy(
                                v_sb.rearrange("p n (h e) -> p n h e", h=HL)
                                [:, ti, :, 0:D],
                                vt[sub][:, 0:DL]
                                .rearrange("p (h e) -> p h e", e=D))

            # wo loads overlap attention (issued after x in DMA queue)
            wo_sb = pp.tile([P, KC, C], BF16)
            nc.sync.dma_start(wo_sb[:], wo.rearrange("(k p) n -> p k n", p=P))

            # per-chunk A2A buffers
            a2a_in = dp.tile([NQ, 2 * G * 2 * P, P], BF16)
            a2a_out = dp.tile([NQ, 2 * G * 2 * P, P], BF16)

            with (
                tc.tile_pool(name="ps_sT", bufs=2, space="PSUM") as ps_sT,
                tc.tile_pool(name="ps_pv", bufs=2, space="PSUM") as ps_pv,
                tc.tile_pool(name="ps_proj", bufs=2, space="PSUM") as ps_proj,
            ):
                def outproj_jobs(qc, src):
                    jobs = []

                    def send():
                        # duplicate real data into both rank halves
                        for rh in range(2):
                            nc.sync.dma_start(
                                a2a_in[qc, rh * G * 2 * P:(rh + 1) * G * 2 * P]
                                .rearrange("(b p) t -> p b t", p=P),
                                src[:].rearrange("p (b t) -> p b t", t=P))

                    def trigger():
                        nc.gpsimd.collective_compute(
                            "AllToAll", mybir.AluOpType.bypass,
                            replica_groups=[[0, 1, 2, 3, 4, 5, 6, 7]],
                            ins=[a2a_in[qc]], outs=[a2a_out[qc]])

                    aoA = wk_p.tile([P, KC * P], BF16, name="aoA", tag="aoA",
                                    bufs=2)
                    aoB = wk_p.tile([P, KC * P], BF16, name="aoB", tag="aoB",
                                    bufs=2)
                    aoC = wk_p.tile([P, KC * P], BF16, name="aoC", tag="aoC",
                                    bufs=2)

                    def recv():
                        for rh, t_ in ((0, aoA), (1, aoB)):
                            nc.sync.dma_start(
                                t_[:].rearrange("p (b t) -> p b t", t=P),
                                a2a_out[qc, rh * G * 2 * P:(rh + 1) * G * 2 * P]
                                .rearrange("(b p) t -> p b t", p=P))

                    def combine():
                        # ao = aoA*selA + aoB*selB (exactly one sel is 1)
                        nc.gpsimd.tensor_scalar_mul(aoA[:], aoA[:], selA[:])
                        nc.gpsimd.tensor_scalar_mul(aoB[:], aoB[:], selB[:])
                        nc.gpsimd.tensor_add(aoC[:], aoA[:], aoB[:])

                    o_sb = wk_p.tile([P, C], F32, name="o_sb", tag="o_sb",
                                     bufs=2)

                    def proj(ni):
                        ps = ps_proj.tile([P, QB], F32, name="proj_ps",
                                          tag="proj_ps")
                        for kk in range(KC):
                            nc.tensor.matmul(
                                ps[:],
                                aoC[:, kk * P:(kk + 1) * P],
                                wo_sb[:, kk, ni * QB:(ni + 1) * QB],
                                start=(kk == 0), stop=(kk == KC - 1))
                        nc.vector.tensor_add(
                            o_sb[:, ni * QB:(ni + 1) * QB], ps[:],
                            bo_bc[:, ni * QB:(ni + 1) * QB])

                    def store():
                        nc.sync.dma_start(out[qc], o_sb[:])

                    jobs = [send, trigger, recv, combine,
                            lambda: proj(0), lambda: proj(1), store]
                    return jobs

                pending = []

                def drain():
                    if pending:
                        pending.pop(0)()

                for qc in range(NQ):
                    nkb = 4 * qc + 4
                    qsl = slice(qc * QB, (qc + 1) * QB)
                    # per-chunk a2a staging tile, written by normalization
                    src = wk_p.tile([P, 2 * G * P], BF16, name="a2a_src",
                                    tag="a2a_src", bufs=2)
                    for mi in range(2):
                        pv_e = ps_pv.tile([P, QB], F32, name="pv", tag="pv")
                        pv_o = ps_pv.tile([P, QB], F32, name="pv", tag="pv")
                        pvs = (pv_e, pv_o)
                        prev = None  # deferred pv emission for pipelining

                        def emit_pv(pkb, pq0, ppT, pqw):
                            for par in range(2):
                                h = 2 * mi + par
                                nc.tensor.matmul(
                                    pvs[par][0:D + 1, pq0:QB],
                                    v_sb[:, pkb,
                                         h * (D + 1):(h + 1) * (D + 1)],
                                    ppT[:, par * QB:par * QB + pqw],
                                    start=(pkb == 0), stop=(pkb == nkb - 1))
                            drain()

                        for kb in range(nkb):
                            di = kb - 4 * qc
                            q0 = max(di, 0) * P
                            qw = QB - q0
                            sT = ps_sT.tile([P, 2 * QB], F32, name="sT",
                                            tag="sT")
                            # head parity par at bank-aligned offset par*QB
                            for par in range(2):
                                nc.tensor.matmul(
                                    sT[:, par * QB:par * QB + qw],
                                    kT_sb[:, par, mi, kb * P:(kb + 1) * P],
                                    qT_sb[:, mi, qc * QB + q0:(qc + 1) * QB],
                                    start=True, stop=True)
                            if prev is not None:
                                emit_pv(*prev)
                            pT = wk_p.tile([P, 2 * QB], BF16, name="pT",
                                           tag="pT")
                            if di >= 0:
                                nc.vector.tensor_add(
                                    sT[:, 0:P], sT[:, 0:P], masks[:])
                                nc.vector.tensor_add(
                                    sT[:, QB:QB + P], sT[:, QB:QB + P],
                                    masks[:])
                                for par in range(2):
                                    nc.scalar.activation(
                                        pT[:, par * QB:par * QB + qw],
                                        sT[:, par * QB:par * QB + qw],
                                        mybir.ActivationFunctionType.Exp,
                                        scale=SCALE)
                            else:
                                nc.scalar.activation(
                                    pT[:], sT[:],
                                    mybir.ActivationFunctionType.Exp,
                                    scale=SCALE)
                            prev = (kb, q0, pT, qw)
                        emit_pv(*prev)

                        # normalize, writing straight into a2a staging layout
                        for par in range(2):
                            lrow = wk_p.tile([1, QB], F32, name="lrow",
                                             tag="lrow", bufs=2)
                            nc.scalar.copy(lrow[:], pvs[par][D:D + 1, :])
                            rbc = wk_p.tile([D, QB], F32, name="rbc",
                                            tag="rbc", bufs=2)
                            nc.vector.reciprocal_approx_fast(
                                out=rbc[0:1, :], in_=lrow[:])
                            nc.gpsimd.partition_broadcast(rbc[:], rbc[0:1, :])
                            nc.vector.tensor_mul(
                                src.rearrange("p (j m t) -> p j m t", j=G,
                                              m=2)
                                [par * D:(par + 1) * D, :, mi, :],
                                pvs[par][0:D, :]
                                .rearrange("e (j t) -> e j t", j=G),
                                rbc[:].rearrange("e (j t) -> e j t", j=G))
                    assert not pending
                    pending = outproj_jobs(qc, src)

                while pending:
                    drain()

                if dbg:
                    nc.sync.dma_start(dbg_q[:], qT_sb[:])
                    nc.sync.dma_start(dbg_k[:], kT_sb[:])
                    nc.sync.dma_start(dbg_v[:], v_sb[:])
                    nc.sync.dma_start(dbg_ai[:], a2a_in[:])
                    nc.sync.dma_start(dbg_ao[:], a2a_out[:])

    nc.compile()
    return nc


def _wo_perm_rows():
    # global dim r = i*256 + mi*128 + parity*64 + d  (i = peer in group)
    # maps to original w_out row (4*i + 2*mi + parity)*64 + d
    r = np.arange(C)
    i, rem = r // 256, r % 256
    mi, rem2 = rem // 128, rem % 128
    par, d_ = rem2 // 64, rem2 % 64
    return (4 * i + 2 * mi + par) * 64 + d_


def _in_maps(x, w_qkv, b_qkv, w_out, b_out):
    xTs = [np.ascontiguousarray(x[b_].T.astype(BF)) for b_ in range(B)]
    bo = np.ascontiguousarray(b_out[None, :].astype(np.float32))
    wo_p = np.ascontiguousarray(w_out[_wo_perm_rows(), :].astype(BF))
    in_maps = []
    for c in range(8):
        b_, g = c // 4, c % 4
        sl = slice(g * DL, (g + 1) * DL)
        selv = np.array([[1.0, 0.0]] if b_ == 0 else [[0.0, 1.0]],
                        dtype=np.float32)
        in_maps.append({
            "xT": xTs[b_],
            "wq": np.ascontiguousarray(w_qkv[:, 0 * C:1 * C][:, sl].astype(BF)),
            "wk": np.ascontiguousarray(w_qkv[:, 1 * C:2 * C][:, sl].astype(BF)),
            "wv": np.ascontiguousarray(w_qkv[:, 2 * C:3 * C][:, sl].astype(BF)),
            "bq": np.ascontiguousarray(
                b_qkv[0 * C:1 * C][sl][None, :].astype(np.float32)),
            "bk": np.ascontiguousarray(
                b_qkv[1 * C:2 * C][sl][None, :].astype(np.float32)),
            "bv": np.ascontiguousarray(
                b_qkv[2 * C:3 * C][sl][None, :].astype(BF)),
            "wo": wo_p,
            "bo": bo,
            "sel": selv,
        })
    return in_maps


def kernel(x, w_qkv, b_qkv, w_out, b_out):
    x = np.ascontiguousarray(np.asarray(x, dtype=np.float32))
    w_qkv = np.asarray(w_qkv, dtype=np.float32)
    b_qkv = np.asarray(b_qkv, dtype=np.float32)
    w_out = np.ascontiguousarray(np.asarray(w_out, dtype=np.float32))
    b_out = np.asarray(b_out, dtype=np.float32)

    if "nc" not in _CACHED:
        _CACHED["nc"] = _build()
    nc = _CACHED["nc"]

    in_maps = _in_maps(x, w_qkv, b_qkv, w_out, b_out)
    res = run_bass_kernel_spmd(nc, in_maps, list(range(8)))
    out_full = np.empty((B, T, C), dtype=np.float32)
    for c in range(8):
        b_, g = c // 4, c % 4
        o = res.results[c]["out"]          # [NQ, P, C]
        for qc in range(NQ):
            r0 = qc * QB + g * P
            out_full[b_, r0:r0 + P, :] = o[qc]
    return out_full


# revision 9
# speedup vs baseline: 1.2955x; 1.2955x over previous
"""Distributed causal self-attention for 8 Trainium2 NeuronCores.

Problem: x[2,2048,1024] @ w_qkv[1024,3072] -> causal MHA (16 heads, d=64)
         -> @ w_out[1024,1024]. All fp32 in/out.

Sharding: core c (0..7) handles batch b=c//4 and head group g=c%4 (4 heads).
Each core projects qkv for its heads (inputs host-cast to bf16), runs flash
attention with transposed-score layout (keys on partitions), then a per-
query-chunk AllToAll over all 8 cores redistributes attention outputs from
head-parallel to token-parallel; each core then applies the full out
projection for its 128-token slice of the chunk.

All matmuls bf16 (fp32 PSUM accumulation); scores use zero-padded
128-contraction (64-contraction matmuls are ~1.8x slower per column on
TRN2). Softmax exp on the ACT engine in fp32->bf16.
"""

import sys

for _p in ("/opt/trn_rl_repo", "/root/.axon_site/_ro/trn_rl_repo"):
    if _p not in sys.path:
        sys.path.insert(0, _p)

import ml_dtypes
import numpy as np

import concourse.bass as bass  # noqa: F401
import concourse.mybir as mybir
import concourse.tile as tile
from concourse import bacc
from concourse.bass_utils import run_bass_kernel_spmd

P = 128
B, T, C = 2, 2048, 1024
H, D = 16, 64
HL = 4               # heads per core
DL = HL * D          # 256 local head dims
KC = C // P          # 8 contraction tiles over C
QB = 512             # query chunk
NQ = T // QB         # 4 query chunks
NT = T // P          # 16 token tiles
G = 4                # cores per batch group
SCALE = 1.0 / 8.0    # 1/sqrt(64)
NEG = -1.0e30

F32 = mybir.dt.float32
BF16 = mybir.dt.bfloat16
BF = ml_dtypes.bfloat16

_CACHED = {}


def _mask_data():
    # tril mask: 0 where key j <= query i, NEG above the diagonal
    j = np.arange(P)[:, None]
    i = np.arange(P)[None, :]
    return np.where(j <= i, 0.0, NEG).astype(np.float32)


def _build(dbg=False):
    nc = bacc.Bacc("TRN2", target_bir_lowering=False, debug=False,
                   num_devices=8)

    # all weights/activations host-cast to bf16
    xT = nc.dram_tensor("xT", [C, T], BF16, kind="ExternalInput")
    wq = nc.dram_tensor("wq", [C, DL], BF16, kind="ExternalInput")
    wk = nc.dram_tensor("wk", [C, DL], BF16, kind="ExternalInput")
    wv = nc.dram_tensor("wv", [C, DL], BF16, kind="ExternalInput")
    bq = nc.dram_tensor("bq", [1, DL], F32, kind="ExternalInput")
    bk = nc.dram_tensor("bk", [1, DL], F32, kind="ExternalInput")
    bv = nc.dram_tensor("bv", [1, DL], BF16, kind="ExternalInput")
    wo = nc.dram_tensor("wo", [C, C], BF16, kind="ExternalInput")  # permuted
    bo = nc.dram_tensor("bo", [1, C], F32, kind="ExternalInput")
    sel = nc.dram_tensor("sel", [1, 2], F32, kind="ExternalInput")
    out = nc.dram_tensor("out", [NQ, P, C], F32, kind="ExternalOutput")
    if dbg:
        dbg_q = nc.dram_tensor("dbg_q", [P, 2, T], BF16, kind="ExternalOutput")
        dbg_k = nc.dram_tensor("dbg_k", [P, 2, 2, T], BF16,
                               kind="ExternalOutput")
        dbg_v = nc.dram_tensor("dbg_v", [P, NT, HL * (D + 1)], BF16,
                               kind="ExternalOutput")
        dbg_ai = nc.dram_tensor("dbg_ai", [NQ, 2 * G * 2 * P, P], BF16,
                                kind="ExternalOutput")
        dbg_ao = nc.dram_tensor("dbg_ao", [NQ, 2 * G * 2 * P, P], BF16,
                                kind="ExternalOutput")

    masks_dram = nc.inline_tensor(_mask_data(), name="cmasks")

    with tile.TileContext(nc) as tc:
        with (
            tc.tile_pool(name="const", bufs=1) as cp,
            tc.tile_pool(name="persist", bufs=1) as pp,
            tc.tile_pool(name="work", bufs=3) as wk_p,
            tc.tile_pool(name="dram", bufs=1, space="DRAM") as dp,
        ):
            # ---- small constants (issued first so weights DMA next) ----
            bq_col = cp.tile([P, 2], F32)
            bk_col = cp.tile([P, 2], F32)
            nc.sync.dma_start(
                bq_col[:], bq[0, :].rearrange("(m p) -> p m", p=P))
            nc.sync.dma_start(
                bk_col[:], bk[0, :].rearrange("(m p) -> p m", p=P))
            bv_sb = cp.tile([1, DL], BF16)
            nc.sync.dma_start(bv_sb[:], bv[:])
            ones_bf = cp.tile([1, P], BF16)
            nc.vector.memset(ones_bf[:], 1.0)

            # ---- weights then x, interleaved per k-tile so the PE can
            # start as soon as (wq0, wk0, x0) land ----
            wq_sb = pp.tile([P, KC, DL], BF16)
            wk_sb = pp.tile([P, KC, DL], BF16)
            wv_sb = pp.tile([P, KC, DL], BF16)

            # ---- persistent activations (bf16) ----
            # qT_pair[mi]: [128 = even-head d | odd-head d, tokens]
            qT_sb = pp.tile([P, 2, T], BF16)
            # kT_pad[parity]: zero-padded to 128 contraction rows
            kT_sb = pp.tile([P, 2, 2, T], BF16)   # [p, parity, mi, t]
            v_sb = pp.tile([P, NT, HL * (D + 1)], BF16)  # per head: 64 v + 1

            nc.vector.memset(kT_sb[64:P, 0], 0.0)
            nc.vector.memset(kT_sb[0:64, 1], 0.0)
            vones = v_sb.rearrange("p n (h e) -> p n h e", h=HL)[:, :, :,
                                                               D:D + 1]
            nc.vector.memset(vones, 1.0)

            with tc.tile_pool(name="xw", bufs=1) as xw:
                xb = xw.tile([P, KC, T], BF16)
                for kk in range(KC):
                    for w_d, w_s in ((wq, wq_sb), (wk, wk_sb)):
                        nc.sync.dma_start(
                            w_s[:, kk, :],
                            w_d.rearrange("(k p) m -> k p m", p=P)[kk])
                    nc.sync.dma_start(
                        xb[:, kk, :],
                        xT.rearrange("(k p) t -> k p t", p=P)[kk])
                for kk in range(KC):
                    nc.sync.dma_start(
                        wv_sb[:, kk, :],
                        wv.rearrange("(k p) m -> k p m", p=P)[kk])

                masks = cp.tile([P, P], F32)
                nc.sync.dma_start(masks[:], masks_dram[:])
                sel_sb = cp.tile([1, 2], F32)
                nc.sync.dma_start(sel_sb[:], sel[:])
                selA = cp.tile([P, 1], F32)
                selB = cp.tile([P, 1], F32)
                nc.gpsimd.partition_broadcast(selA[:], sel_sb[:, 0:1])
                nc.gpsimd.partition_broadcast(selB[:], sel_sb[:, 1:2])
                bo_sb = cp.tile([1, C], F32)
                nc.sync.dma_start(bo_sb[:], bo[:])
                bo_bc = cp.tile([P, C], F32)
                nc.gpsimd.partition_broadcast(bo_bc[:], bo_sb[:])

                # ---- phase A: qkv projection, kk-major over 8 psum banks --
                with tc.tile_pool(name="ps_a", bufs=8, space="PSUM") as psa:
                    # stage 1+2: q/k for ni pairs, kk-major streaming vs DMA
                    for nh in range(2):          # ni half: (0,1) then (2,3)
                        groups = []
                        for (w_sb, b_col, dst) in ((wq_sb, bq_col, 0),
                                                   (wk_sb, bk_col, 1)):
                            for mi in range(2):
                                for ni in (2 * nh, 2 * nh + 1):
                                    groups.append((w_sb, b_col, dst, mi, ni))
                        tiles = {}
                        for gi, (w_sb, b_col, dst, mi, ni) in enumerate(groups):
                            tiles[gi] = psa.tile([P, QB], F32, name="ps_qk",
                                                 tag="ps_a")
                        for kk in range(KC):
                            for gi, (w_sb, b_col, dst, mi, ni) in enumerate(
                                    groups):
                                nc.tensor.matmul(
                                    tiles[gi][:],
                                    w_sb[:, kk, mi * P:(mi + 1) * P],
                                    xb[:, kk, ni * QB:(ni + 1) * QB],
                                    start=(kk == 0), stop=(kk == KC - 1))
                        for gi, (w_sb, b_col, dst, mi, ni) in enumerate(
                                groups):
                            ps = tiles[gi]
                            tsl = slice(ni * QB, (ni + 1) * QB)
                            if dst == 0:     # q: straight copy, pair-stacked
                                nc.vector.tensor_scalar_add(
                                    qT_sb[:, mi, tsl], ps[:],
                                    b_col[:, mi:mi + 1])
                            else:            # k: split halves into padded kT
                                nc.vector.tensor_scalar_add(
                                    kT_sb[0:64, 0, mi, tsl], ps[0:64, :],
                                    b_col[0:64, mi:mi + 1])
                                nc.vector.tensor_scalar_add(
                                    kT_sb[64:P, 1, mi, tsl], ps[64:P, :],
                                    b_col[64:P, mi:mi + 1])
                    # stage 3: v, [tok, dims] layout; one group per psum bank
                    # (start=True zeroes the whole bank, so never co-locate
                    # two accumulation groups in one bank)
                    for vp in range(2):
                        vt = {}
                        for sub in range(NT // 2):
                            vt[sub] = psa.tile([P, QB], F32, name="ps_v",
                                               tag="ps_a")
                        for kk in range(KC):
                            for sub in range(NT // 2):
                                ti = vp * (NT // 2) + sub
                                nc.tensor.matmul(
                                    vt[sub][:, 0:DL],
                                    xb[:, kk, ti * P:(ti + 1) * P],
                                    wv_sb[:, kk, :],
                                    start=(kk == 0), stop=False)
                        for sub in range(NT // 2):
                            nc.tensor.matmul(
                                vt[sub][:, 0:DL],
                                ones_bf[:, 0:P], bv_sb[:],
                                start=False, stop=True)
                        for sub in range(NT // 2):
                            ti = vp * (NT // 2) + sub
                            nc.vector.tensor_copy(
                                v_sb.rearrange("p n (h e) -> p n h e", h=HL)
                                [:, ti, :, 0:D],
                                vt[sub][:, 0:DL]
                                .rearrange("p (h e) -> p h e", e=D))

            # wo loads overlap attention (issued after x in DMA queue)
            wo_sb = pp.tile([P, KC, C], BF16)
            nc.sync.dma_start(wo_sb[:], wo.rearrange("(k p) n -> p k n", p=P))

            # per-chunk A2A buffers
            a2a_in = dp.tile([NQ, 2 * G * 2 * P, P], BF16)
            a2a_out = dp.tile([NQ, 2 * G * 2 * P, P], BF16)

            with (
                tc.tile_pool(name="ps_sT", bufs=2, space="PSUM") as ps_sT,
                tc.tile_pool(name="ps_pv", bufs=2, space="PSUM") as ps_pv,
                tc.tile_pool(name="ps_proj", bufs=2, space="PSUM") as ps_proj,
            ):
                def outproj_jobs(qc, src):
                    jobs = []

                    def send():
                        # duplicate real data into both rank halves
                        for rh in range(2):
                            nc.sync.dma_start(
                                a2a_in[qc, rh * G * 2 * P:(rh + 1) * G * 2 * P]
                                .rearrange("(b p) t -> p b t", p=P),
                                src[:].rearrange("p (b t) -> p b t", t=P))

                    def trigger():
                        nc.gpsimd.collective_compute(
                            "AllToAll", mybir.AluOpType.bypass,
                            replica_groups=[[0, 1, 2, 3, 4, 5, 6, 7]],
                            ins=[a2a_in[qc]], outs=[a2a_out[qc]])

                    aoA = wk_p.tile([P, KC * P], BF16, name="aoA", tag="aoA",
                                    bufs=2)
                    aoB = wk_p.tile([P, KC * P], BF16, name="aoB", tag="aoB",
                                    bufs=2)
                    aoC = wk_p.tile([P, KC * P], BF16, name="aoC", tag="aoC",
                                    bufs=2)

                    def recv():
                        for rh, t_ in ((0, aoA), (1, aoB)):
                            nc.sync.dma_start(
                                t_[:].rearrange("p (b t) -> p b t", t=P),
                                a2a_out[qc, rh * G * 2 * P:(rh + 1) * G * 2 * P]
                                .rearrange("(b p) t -> p b t", p=P))

                    def combine():
                        # ao = aoA*selA + aoB*selB (exactly one sel is 1)
                        nc.vector.tensor_scalar_mul(aoA[:], aoA[:], selA[:])
                        nc.vector.tensor_scalar_mul(aoB[:], aoB[:], selB[:])
                        nc.vector.tensor_add(aoC[:], aoA[:], aoB[:])

                    o_sb = wk_p.tile([P, C], F32, name="o_sb", tag="o_sb",
                                     bufs=2)

                    def proj(ni):
                        ps = ps_proj.tile([P, QB], F32, name="proj_ps",
                                          tag="proj_ps")
                        for kk in range(KC):
                            nc.tensor.matmul(
                                ps[:],
                                aoC[:, kk * P:(kk + 1) * P],
                                wo_sb[:, kk, ni * QB:(ni + 1) * QB],
                                start=(kk == 0), stop=(kk == KC - 1))
                        nc.vector.tensor_add(
                            o_sb[:, ni * QB:(ni + 1) * QB], ps[:],
                            bo_bc[:, ni * QB:(ni + 1) * QB])

                    def store():
                        nc.sync.dma_start(out[qc], o_sb[:])

                    jobs = [send, trigger, recv, combine,
                            lambda: proj(0), lambda: proj(1), store]
                    return jobs

                pending = []

                def drain():
                    if pending:
                        pending.pop(0)()

                for qc in range(NQ):
                    nkb = 4 * qc + 4
                    qsl = slice(qc * QB, (qc + 1) * QB)
                    # per-chunk a2a staging tile, written by normalization
                    src = wk_p.tile([P, 2 * G * P], BF16, name="a2a_src",
                                    tag="a2a_src", bufs=2)
                    for mi in range(2):
                        pv_e = ps_pv.tile([P, QB], F32, name="pv", tag="pv")
                        pv_o = ps_pv.tile([P, QB], F32, name="pv", tag="pv")
                        pvs = (pv_e, pv_o)
                        prev = None  # deferred pv emission for pipelining

                        def emit_pv(pkb, pq0, ppT, pqw):
                            for par in range(2):
                                h = 2 * mi + par
                                nc.tensor.matmul(
                                    pvs[par][0:D + 1, pq0:QB],
                                    v_sb[:, pkb,
                                         h * (D + 1):(h + 1) * (D + 1)],
                                    ppT[:, par * QB:par * QB + pqw],
                                    start=(pkb == 0), stop=(pkb == nkb - 1))
                            drain()

                        for kb in range(nkb):
                            di = kb - 4 * qc
                            q0 = max(di, 0) * P
                            qw = QB - q0
                            sT = ps_sT.tile([P, 2 * QB], F32, name="sT",
                                            tag="sT")
                            # head parity par at bank-aligned offset par*QB
                            for par in range(2):
                                nc.tensor.matmul(
                                    sT[:, par * QB:par * QB + qw],
                                    kT_sb[:, par, mi, kb * P:(kb + 1) * P],
                                    qT_sb[:, mi, qc * QB + q0:(qc + 1) * QB],
                                    start=True, stop=True)
                            if prev is not None:
                                emit_pv(*prev)
                            pT = wk_p.tile([P, 2 * QB], BF16, name="pT",
                                           tag="pT")
                            if di >= 0:
                                nc.vector.tensor_add(
                                    sT[:, 0:P], sT[:, 0:P], masks[:])
                                nc.vector.tensor_add(
                                    sT[:, QB:QB + P], sT[:, QB:QB + P],
                                    masks[:])
                                for par in range(2):
                                    nc.scalar.activation(
                                        pT[:, par * QB:par * QB + qw],
                                        sT[:, par * QB:par * QB + qw],
                                        mybir.ActivationFunctionType.Exp,
                                        scale=SCALE)
                            else:
                                nc.scalar.activation(
                                    pT[:], sT[:],
                                    mybir.ActivationFunctionType.Exp,
                                    scale=SCALE)
                            prev = (kb, q0, pT, qw)
                        emit_pv(*prev)

                        # normalize, writing straight into a2a staging layout
                        for par in range(2):
                            lrow = wk_p.tile([1, QB], F32, name="lrow",
                                             tag="lrow", bufs=2)
                            nc.scalar.copy(lrow[:], pvs[par][D:D + 1, :])
                            rbc = wk_p.tile([D, QB], F32, name="rbc",
                                            tag="rbc", bufs=2)
                            nc.vector.reciprocal_approx_fast(
                                out=rbc[0:1, :], in_=lrow[:])
                            nc.gpsimd.partition_broadcast(rbc[:], rbc[0:1, :])
                            nc.vector.tensor_mul(
                                src.rearrange("p (j m t) -> p j m t", j=G,
                                              m=2)
                                [par * D:(par + 1) * D, :, mi, :],
                                pvs[par][0:D, :]
                                .rearrange("e (j t) -> e j t", j=G),
                                rbc[:].rearrange("e (j t) -> e j t", j=G))
                    assert not pending
                    pending = outproj_jobs(qc, src)

                while pending:
                    drain()

                if dbg:
                    nc.sync.dma_start(dbg_q[:], qT_sb[:])
                    nc.sync.dma_start(dbg_k[:], kT_sb[:])
                    nc.sync.dma_start(dbg_v[:], v_sb[:])
                    nc.sync.dma_start(dbg_ai[:], a2a_in[:])
                    nc.sync.dma_start(dbg_ao[:], a2a_out[:])

    nc.compile()
    return nc


def _wo_perm_rows():
    # global dim r = i*256 + mi*128 + parity*64 + d  (i = peer in group)
    # maps to original w_out row (4*i + 2*mi + parity)*64 + d
    r = np.arange(C)
    i, rem = r // 256, r % 256
    mi, rem2 = rem // 128, rem % 128
    par, d_ = rem2 // 64, rem2 % 64
    return (4 * i + 2 * mi + par) * 64 + d_


def _in_maps(x, w_qkv, b_qkv, w_out, b_out):
    xTs = [np.ascontiguousarray(x[b_].T.astype(BF)) for b_ in range(B)]
    bo = np.ascontiguousarray(b_out[None, :].astype(np.float32))
    wo_p = np.ascontiguousarray(w_out[_wo_perm_rows(), :].astype(BF))
    in_maps = []
    for c in range(8):
        b_, g = c // 4, c % 4
        sl = slice(g * DL, (g + 1) * DL)
        selv = np.array([[1.0, 0.0]] if b_ == 0 else [[0.0, 1.0]],
                        dtype=np.float32)
        in_maps.append({
            "xT": xTs[b_],
            "wq": np.ascontiguousarray(w_qkv[:, 0 * C:1 * C][:, sl].astype(BF)),
            "wk": np.ascontiguousarray(w_qkv[:, 1 * C:2 * C][:, sl].astype(BF)),
            "wv": np.ascontiguousarray(w_qkv[:, 2 * C:3 * C][:, sl].astype(BF)),
            "bq": np.ascontiguousarray(
                b_qkv[0 * C:1 * C][sl][None, :].astype(np.float32)),
            "bk": np.ascontiguousarray(
                b_qkv[1 * C:2 * C][sl][None, :].astype(np.float32)),
            "bv": np.ascontiguousarray(
                b_qkv[2 * C:3 * C][sl][None, :].astype(BF)),
            "wo": wo_p,
            "bo": bo,
            "sel": selv,
        })
    return in_maps


def kernel(x, w_qkv, b_qkv, w_out, b_out):
    x = np.ascontiguousarray(np.asarray(x, dtype=np.float32))
    w_qkv = np.asarray(w_qkv, dtype=np.float32)
    b_qkv = np.asarray(b_qkv, dtype=np.float32)
    w_out = np.ascontiguousarray(np.asarray(w_out, dtype=np.float32))
    b_out = np.asarray(b_out, dtype=np.float32)

    if "nc" not in _CACHED:
        _CACHED["nc"] = _build()
    nc = _CACHED["nc"]

    in_maps = _in_maps(x, w_qkv, b_qkv, w_out, b_out)
    res = run_bass_kernel_spmd(nc, in_maps, list(range(8)))
    out_full = np.empty((B, T, C), dtype=np.float32)
    for c in range(8):
        b_, g = c // 4, c % 4
        o = res.results[c]["out"]          # [NQ, P, C]
        for qc in range(NQ):
            r0 = qc * QB + g * P
            out_full[b_, r0:r0 + P, :] = o[qc]
    return out_full


# revision 10
# speedup vs baseline: 1.5859x; 1.2241x over previous
"""Distributed causal self-attention for 8 Trainium2 NeuronCores.

Problem: x[2,2048,1024] @ w_qkv[1024,3072] -> causal MHA (16 heads, d=64)
         -> @ w_out[1024,1024]. All fp32 in/out.

Sharding: core c (0..7) handles batch b=c//4 and head group g=c%4 (4 heads).
Each core projects qkv for its heads (inputs host-cast to bf16), runs flash
attention with transposed-score layout (keys on partitions), then a per-
query-chunk AllToAll over all 8 cores redistributes attention outputs from
head-parallel to token-parallel; each core then applies the full out
projection for its 128-token slice of the chunk.

All matmuls bf16 (fp32 PSUM accumulation); scores use zero-padded
128-contraction (64-contraction matmuls are ~1.8x slower per column on
TRN2). Softmax exp on the ACT engine in fp32->bf16.
"""

import sys

for _p in ("/opt/trn_rl_repo", "/root/.axon_site/_ro/trn_rl_repo"):
    if _p not in sys.path:
        sys.path.insert(0, _p)

import ml_dtypes
import numpy as np

import concourse.bass as bass  # noqa: F401
import concourse.mybir as mybir
import concourse.tile as tile
from concourse import bacc
from concourse.bass_utils import run_bass_kernel_spmd

P = 128
B, T, C = 2, 2048, 1024
H, D = 16, 64
HL = 4               # heads per core
DL = HL * D          # 256 local head dims
KC = C // P          # 8 contraction tiles over C
QB = 512             # query chunk
NQ = T // QB         # 4 query chunks
NT = T // P          # 16 token tiles
G = 4                # cores per batch group
SCALE = 1.0 / 8.0    # 1/sqrt(64)
NEG = -1.0e30

F32 = mybir.dt.float32
BF16 = mybir.dt.bfloat16
BF = ml_dtypes.bfloat16

_CACHED = {}


def _mask_data():
    # tril mask: 0 where key j <= query i, NEG above the diagonal
    j = np.arange(P)[:, None]
    i = np.arange(P)[None, :]
    return np.where(j <= i, 0.0, NEG).astype(np.float32)


def _build(dbg=False):
    nc = bacc.Bacc("TRN2", target_bir_lowering=False, debug=False,
                   num_devices=8)

    # all weights/activations host-cast to bf16
    xT = nc.dram_tensor("xT", [C, T], BF16, kind="ExternalInput")
    wq = nc.dram_tensor("wq", [C, DL], BF16, kind="ExternalInput")
    wk = nc.dram_tensor("wk", [C, DL], BF16, kind="ExternalInput")
    wv = nc.dram_tensor("wv", [C, DL], BF16, kind="ExternalInput")
    bq = nc.dram_tensor("bq", [1, DL], F32, kind="ExternalInput")
    bk = nc.dram_tensor("bk", [1, DL], F32, kind="ExternalInput")
    bv = nc.dram_tensor("bv", [1, DL], BF16, kind="ExternalInput")
    wo = nc.dram_tensor("wo", [C, C], BF16, kind="ExternalInput")  # permuted
    bo = nc.dram_tensor("bo", [1, C], F32, kind="ExternalInput")
    sel = nc.dram_tensor("sel", [1, 2], F32, kind="ExternalInput")
    out = nc.dram_tensor("out", [NQ, P, C], F32, kind="ExternalOutput")
    if dbg:
        dbg_q = nc.dram_tensor("dbg_q", [P, 2, T], BF16, kind="ExternalOutput")
        dbg_k = nc.dram_tensor("dbg_k", [P, 2, 2, T], BF16,
                               kind="ExternalOutput")
        dbg_v = nc.dram_tensor("dbg_v", [P, NT, HL * (D + 1)], BF16,
                               kind="ExternalOutput")
        dbg_ai = nc.dram_tensor("dbg_ai", [NQ, 2 * G * 2 * P, P], BF16,
                                kind="ExternalOutput")
        dbg_ao = nc.dram_tensor("dbg_ao", [NQ, 2 * G * 2 * P, P], BF16,
                                kind="ExternalOutput")

    masks_dram = nc.inline_tensor(_mask_data(), name="cmasks")

    warm_in = nc.dram_tensor("warm_in", [8, P], BF16, kind="Internal")
    warm_out = nc.dram_tensor("warm_out", [8, P], BF16, kind="Internal")

    with tile.TileContext(nc) as tc:
        with (
            tc.tile_pool(name="const", bufs=1) as cp,
            tc.tile_pool(name="persist", bufs=1) as pp,
            tc.tile_pool(name="work", bufs=3) as wk_p,
            tc.tile_pool(name="dram", bufs=1, space="DRAM") as dp,
        ):
            # warm-up collective: runs the one-time global barrier + CC-core
            # init concurrently with phase A instead of stalling chunk 0's
            # real AllToAll
            nc.gpsimd.collective_compute(
                "AllToAll", mybir.AluOpType.bypass,
                replica_groups=[[0, 1, 2, 3, 4, 5, 6, 7]],
                ins=[warm_in[:]], outs=[warm_out[:]])
            # ---- small constants (issued first so weights DMA next) ----
            bq_col = cp.tile([P, 2], F32)
            bk_col = cp.tile([P, 2], F32)
            nc.sync.dma_start(
                bq_col[:], bq[0, :].rearrange("(m p) -> p m", p=P))
            nc.sync.dma_start(
                bk_col[:], bk[0, :].rearrange("(m p) -> p m", p=P))
            bv_sb = cp.tile([1, DL], BF16)
            nc.sync.dma_start(bv_sb[:], bv[:])
            ones_bf = cp.tile([1, P], BF16)
            nc.vector.memset(ones_bf[:], 1.0)

            # ---- weights then x, interleaved per k-tile so the PE can
            # start as soon as (wq0, wk0, x0) land ----
            wq_sb = pp.tile([P, KC, DL], BF16)
            wk_sb = pp.tile([P, KC, DL], BF16)
            wv_sb = pp.tile([P, KC, DL], BF16)

            # ---- persistent activations (bf16) ----
            # qT_pair[mi]: [128 = even-head d | odd-head d, tokens]
            qT_sb = pp.tile([P, 2, T], BF16)
            # kT_pad[parity]: zero-padded to 128 contraction rows
            kT_sb = pp.tile([P, 2, 2, T], BF16)   # [p, parity, mi, t]
            v_sb = pp.tile([P, NT, HL * (D + 1)], BF16)  # per head: 64 v + 1

            nc.vector.memset(kT_sb[64:P, 0], 0.0)
            nc.vector.memset(kT_sb[0:64, 1], 0.0)
            vones = v_sb.rearrange("p n (h e) -> p n h e", h=HL)[:, :, :,
                                                               D:D + 1]
            nc.vector.memset(vones, 1.0)

            with tc.tile_pool(name="xw", bufs=1) as xw:
                xb = xw.tile([P, KC, T], BF16)
                for kk in range(KC):
                    for w_d, w_s in ((wq, wq_sb), (wk, wk_sb)):
                        nc.sync.dma_start(
                            w_s[:, kk, :],
                            w_d.rearrange("(k p) m -> k p m", p=P)[kk])
                    nc.sync.dma_start(
                        xb[:, kk, :],
                        xT.rearrange("(k p) t -> k p t", p=P)[kk])
                for kk in range(KC):
                    nc.sync.dma_start(
                        wv_sb[:, kk, :],
                        wv.rearrange("(k p) m -> k p m", p=P)[kk])

                masks = cp.tile([P, P], F32)
                nc.sync.dma_start(masks[:], masks_dram[:])
                sel_sb = cp.tile([1, 2], F32)
                nc.sync.dma_start(sel_sb[:], sel[:])
                selA = cp.tile([P, 1], F32)
                selB = cp.tile([P, 1], F32)
                nc.gpsimd.partition_broadcast(selA[:], sel_sb[:, 0:1])
                nc.gpsimd.partition_broadcast(selB[:], sel_sb[:, 1:2])
                bo_sb = cp.tile([1, C], F32)
                nc.sync.dma_start(bo_sb[:], bo[:])
                bo_bc = cp.tile([P, C], F32)
                nc.gpsimd.partition_broadcast(bo_bc[:], bo_sb[:])

                # ---- phase A: qkv projection, kk-major over 8 psum banks --
                with tc.tile_pool(name="ps_a", bufs=8, space="PSUM") as psa:
                    # stage 1+2: q/k for ni pairs, kk-major streaming vs DMA
                    for nh in range(2):          # ni half: (0,1) then (2,3)
                        groups = []
                        for (w_sb, b_col, dst) in ((wq_sb, bq_col, 0),
                                                   (wk_sb, bk_col, 1)):
                            for mi in range(2):
                                for ni in (2 * nh, 2 * nh + 1):
                                    groups.append((w_sb, b_col, dst, mi, ni))
                        tiles = {}
                        for gi, (w_sb, b_col, dst, mi, ni) in enumerate(groups):
                            tiles[gi] = psa.tile([P, QB], F32, name="ps_qk",
                                                 tag="ps_a")
                        for kk in range(KC):
                            for gi, (w_sb, b_col, dst, mi, ni) in enumerate(
                                    groups):
                                nc.tensor.matmul(
                                    tiles[gi][:],
                                    w_sb[:, kk, mi * P:(mi + 1) * P],
                                    xb[:, kk, ni * QB:(ni + 1) * QB],
                                    start=(kk == 0), stop=(kk == KC - 1))
                        for gi, (w_sb, b_col, dst, mi, ni) in enumerate(
                                groups):
                            ps = tiles[gi]
                            tsl = slice(ni * QB, (ni + 1) * QB)
                            if dst == 0:     # q: straight copy, pair-stacked
                                nc.vector.tensor_scalar_add(
                                    qT_sb[:, mi, tsl], ps[:],
                                    b_col[:, mi:mi + 1])
                            else:            # k: split halves into padded kT
                                nc.vector.tensor_scalar_add(
                                    kT_sb[0:64, 0, mi, tsl], ps[0:64, :],
                                    b_col[0:64, mi:mi + 1])
                                nc.vector.tensor_scalar_add(
                                    kT_sb[64:P, 1, mi, tsl], ps[64:P, :],
                                    b_col[64:P, mi:mi + 1])
                    # stage 3: v, [tok, dims] layout; one group per psum bank
                    # (start=True zeroes the whole bank, so never co-locate
                    # two accumulation groups in one bank)
                    for vp in range(2):
                        vt = {}
                        for sub in range(NT // 2):
                            vt[sub] = psa.tile([P, QB], F32, name="ps_v",
                                               tag="ps_a")
                        for kk in range(KC):
                            for sub in range(NT // 2):
                                ti = vp * (NT // 2) + sub
                                nc.tensor.matmul(
                                    vt[sub][:, 0:DL],
                                    xb[:, kk, ti * P:(ti + 1) * P],
                                    wv_sb[:, kk, :],
                                    start=(kk == 0), stop=False)
                        for sub in range(NT // 2):
                            nc.tensor.matmul(
                                vt[sub][:, 0:DL],
                                ones_bf[:, 0:P], bv_sb[:],
                                start=False, stop=True)
                        for sub in range(NT // 2):
                            ti = vp * (NT // 2) + sub
                            nc.vector.tensor_copy(
                                v_sb.rearrange("p n (h e) -> p n h e", h=HL)
                                [:, ti, :, 0:D],
                                vt[sub][:, 0:DL]
                                .rearrange("p (h e) -> p h e", e=D))

            # wo loads overlap attention (issued after x in DMA queue)
            wo_sb = pp.tile([P, KC, C], BF16)
            nc.sync.dma_start(wo_sb[:], wo.rearrange("(k p) n -> p k n", p=P))

            # per-chunk A2A buffers
            a2a_in = dp.tile([NQ, 2 * G * 2 * P, P], BF16)
            a2a_out = dp.tile([NQ, 2 * G * 2 * P, P], BF16)

            with (
                tc.tile_pool(name="ps_sT", bufs=2, space="PSUM") as ps_sT,
                tc.tile_pool(name="ps_pv", bufs=2, space="PSUM") as ps_pv,
                tc.tile_pool(name="ps_proj", bufs=2, space="PSUM") as ps_proj,
            ):
                def outproj_jobs(qc, src):
                    jobs = []

                    def send():
                        # duplicate real data into both rank halves
                        for rh in range(2):
                            nc.sync.dma_start(
                                a2a_in[qc, rh * G * 2 * P:(rh + 1) * G * 2 * P]
                                .rearrange("(b p) t -> p b t", p=P),
                                src[:].rearrange("p (b t) -> p b t", t=P))

                    def trigger():
                        nc.gpsimd.collective_compute(
                            "AllToAll", mybir.AluOpType.bypass,
                            replica_groups=[[0, 1, 2, 3, 4, 5, 6, 7]],
                            ins=[a2a_in[qc]], outs=[a2a_out[qc]])

                    aoA = wk_p.tile([P, KC * P], BF16, name="aoA", tag="aoA",
                                    bufs=2)
                    aoB = wk_p.tile([P, KC * P], BF16, name="aoB", tag="aoB",
                                    bufs=2)
                    aoC = wk_p.tile([P, KC * P], BF16, name="aoC", tag="aoC",
                                    bufs=2)

                    def recv():
                        for rh, t_ in ((0, aoA), (1, aoB)):
                            nc.sync.dma_start(
                                t_[:].rearrange("p (b t) -> p b t", t=P),
                                a2a_out[qc, rh * G * 2 * P:(rh + 1) * G * 2 * P]
                                .rearrange("(b p) t -> p b t", p=P))

                    def combine():
                        # ao = aoA*selA + aoB*selB (exactly one sel is 1)
                        nc.vector.tensor_scalar_mul(aoA[:], aoA[:], selA[:])
                        nc.vector.tensor_scalar_mul(aoB[:], aoB[:], selB[:])
                        nc.vector.tensor_add(aoC[:], aoA[:], aoB[:])

                    o_sb = wk_p.tile([P, C], F32, name="o_sb", tag="o_sb",
                                     bufs=2)

                    def proj(ni):
                        ps = ps_proj.tile([P, QB], F32, name="proj_ps",
                                          tag="proj_ps")
                        for kk in range(KC):
                            nc.tensor.matmul(
                                ps[:],
                                aoC[:, kk * P:(kk + 1) * P],
                                wo_sb[:, kk, ni * QB:(ni + 1) * QB],
                                start=(kk == 0), stop=(kk == KC - 1))
                        nc.vector.tensor_add(
                            o_sb[:, ni * QB:(ni + 1) * QB], ps[:],
                            bo_bc[:, ni * QB:(ni + 1) * QB])

                    def store():
                        nc.sync.dma_start(out[qc], o_sb[:])

                    jobs = [send, trigger, recv, combine,
                            lambda: proj(0), lambda: proj(1), store]
                    return jobs

                pending = []

                def drain():
                    if pending:
                        pending.pop(0)()

                for qc in range(NQ):
                    nkb = 4 * qc + 4
                    qsl = slice(qc * QB, (qc + 1) * QB)
                    # per-chunk a2a staging tile, written by normalization
                    src = wk_p.tile([P, 2 * G * P], BF16, name="a2a_src",
                                    tag="a2a_src", bufs=2)
                    for mi in range(2):
                        pv_e = ps_pv.tile([P, QB], F32, name="pv", tag="pv")
                        pv_o = ps_pv.tile([P, QB], F32, name="pv", tag="pv")
                        pvs = (pv_e, pv_o)
                        prev = None  # deferred pv emission for pipelining

                        def emit_pv(pkb, pq0, ppT, pqw):
                            for par in range(2):
                                h = 2 * mi + par
                                nc.tensor.matmul(
                                    pvs[par][0:D + 1, pq0:QB],
                                    v_sb[:, pkb,
                                         h * (D + 1):(h + 1) * (D + 1)],
                                    ppT[:, par * QB:par * QB + pqw],
                                    start=(pkb == 0), stop=(pkb == nkb - 1))
                            drain()

                        for kb in range(nkb):
                            di = kb - 4 * qc
                            q0 = max(di, 0) * P
                            qw = QB - q0
                            sT = ps_sT.tile([P, 2 * QB], F32, name="sT",
                                            tag="sT")
                            # head parity par at bank-aligned offset par*QB
                            for par in range(2):
                                nc.tensor.matmul(
                                    sT[:, par * QB:par * QB + qw],
                                    kT_sb[:, par, mi, kb * P:(kb + 1) * P],
                                    qT_sb[:, mi, qc * QB + q0:(qc + 1) * QB],
                                    start=True, stop=True)
                            if prev is not None:
                                emit_pv(*prev)
                            pT = wk_p.tile([P, 2 * QB], BF16, name="pT",
                                           tag="pT")
                            if di >= 0:
                                nc.vector.tensor_add(
                                    sT[:, 0:P], sT[:, 0:P], masks[:])
                                nc.vector.tensor_add(
                                    sT[:, QB:QB + P], sT[:, QB:QB + P],
                                    masks[:])
                                for par in range(2):
                                    nc.scalar.activation(
                                        pT[:, par * QB:par * QB + qw],
                                        sT[:, par * QB:par * QB + qw],
                                        mybir.ActivationFunctionType.Exp,
                                        scale=SCALE)
                            else:
                                nc.scalar.activation(
                                    pT[:], sT[:],
                                    mybir.ActivationFunctionType.Exp,
                                    scale=SCALE)
                            prev = (kb, q0, pT, qw)
                        emit_pv(*prev)

                        # normalize, writing straight into a2a staging layout
                        for par in range(2):
                            lrow = wk_p.tile([1, QB], F32, name="lrow",
                                             tag="lrow", bufs=2)
                            nc.scalar.copy(lrow[:], pvs[par][D:D + 1, :])
                            rbc = wk_p.tile([D, QB], F32, name="rbc",
                                            tag="rbc", bufs=2)
                            nc.vector.reciprocal_approx_fast(
                                out=rbc[0:1, :], in_=lrow[:])
                            nc.gpsimd.partition_broadcast(rbc[:], rbc[0:1, :])
                            nc.vector.tensor_mul(
                                src.rearrange("p (j m t) -> p j m t", j=G,
                                              m=2)
                                [par * D:(par + 1) * D, :, mi, :],
                                pvs[par][0:D, :]
                                .rearrange("e (j t) -> e j t", j=G),
                                rbc[:].rearrange("e (j t) -> e j t", j=G))
                    assert not pending
                    pending = outproj_jobs(qc, src)

                while pending:
                    drain()

                if dbg:
                    nc.sync.dma_start(dbg_q[:], qT_sb[:])
                    nc.sync.dma_start(dbg_k[:], kT_sb[:])
                    nc.sync.dma_start(dbg_v[:], v_sb[:])
                    nc.sync.dma_start(dbg_ai[:], a2a_in[:])
                    nc.sync.dma_start(dbg_ao[:], a2a_out[:])

    nc.compile()
    return nc


def _wo_perm_rows():
    # global dim r = i*256 + mi*128 + parity*64 + d  (i = peer in group)
    # maps to original w_out row (4*i + 2*mi + parity)*64 + d
    r = np.arange(C)
    i, rem = r // 256, r % 256
    mi, rem2 = rem // 128, rem % 128
    par, d_ = rem2 // 64, rem2 % 64
    return (4 * i + 2 * mi + par) * 64 + d_


def _in_maps(x, w_qkv, b_qkv, w_out, b_out):
    xTs = [np.ascontiguousarray(x[b_].T.astype(BF)) for b_ in range(B)]
    bo = np.ascontiguousarray(b_out[None, :].astype(np.float32))
    wo_p = np.ascontiguousarray(w_out[_wo_perm_rows(), :].astype(BF))
    in_maps = []
    for c in range(8):
        b_, g = c // 4, c % 4
        sl = slice(g * DL, (g + 1) * DL)
        selv = np.array([[1.0, 0.0]] if b_ == 0 else [[0.0, 1.0]],
                        dtype=np.float32)
        in_maps.append({
            "xT": xTs[b_],
            "wq": np.ascontiguousarray(w_qkv[:, 0 * C:1 * C][:, sl].astype(BF)),
            "wk": np.ascontiguousarray(w_qkv[:, 1 * C:2 * C][:, sl].astype(BF)),
            "wv": np.ascontiguousarray(w_qkv[:, 2 * C:3 * C][:, sl].astype(BF)),
            "bq": np.ascontiguousarray(
                b_qkv[0 * C:1 * C][sl][None, :].astype(np.float32)),
            "bk": np.ascontiguousarray(
                b_qkv[1 * C:2 * C][sl][None, :].astype(np.float32)),
            "bv": np.ascontiguousarray(
                b_qkv[2 * C:3 * C][sl][None, :].astype(BF)),
            "wo": wo_p,
            "bo": bo,
            "sel": selv,
        })
    return in_maps


def kernel(x, w_qkv, b_qkv, w_out, b_out):
    x = np.ascontiguousarray(np.asarray(x, dtype=np.float32))
    w_qkv = np.asarray(w_qkv, dtype=np.float32)
    b_qkv = np.asarray(b_qkv, dtype=np.float32)
    w_out = np.ascontiguousarray(np.asarray(w_out, dtype=np.float32))
    b_out = np.asarray(b_out, dtype=np.float32)

    if "nc" not in _CACHED:
        _CACHED["nc"] = _build()
    nc = _CACHED["nc"]

    in_maps = _in_maps(x, w_qkv, b_qkv, w_out, b_out)
    res = run_bass_kernel_spmd(nc, in_maps, list(range(8)))
    out_full = np.empty((B, T, C), dtype=np.float32)
    for c in range(8):
        b_, g = c // 4, c % 4
        o = res.results[c]["out"]          # [NQ, P, C]
        for qc in range(NQ):
            r0 = qc * QB + g * P
            out_full[b_, r0:r0 + P, :] = o[qc]
    return out_full
